# revision 31
# baseline (speedup 1.0000x reference)
"""MultiHeadSelfAttentionWithLagBias on 8 TRN2 NeuronCores — v3b.

Sharding: tensor-parallel over heads — 16 heads / 8 cores = 2 heads per
core. Each core computes QKV projections for its head slice (full x),
attention with the lag bias for its 2 heads over both batch elements,
and a partial output projection (its 128 rows of wo). Host sums the 8
fp16 partials and adds bo.

v3b vs the 493us f32r baseline (measured 257.6us, rel err 8.8e-4):
  * fp16 datapath end to end (same PE rate as f32r, half the DMA/SBUF
    traffic, 2x DVE mode on elementwise ops; fp8 was tried and fails
    the 2e-2 gate — absolute logit noise ~0.04 maps 1:1 to max-metric
    via peaked-softmax queries — and DoubleRow gave no HW speedup).
    The all-16-bit low-power profile also removes the power throttle:
    the PE sustains ~2.4GHz instead of sagging to ~0.7GHz.
  * Lag bias applied as pe *= exp(bias) AFTER the exp instead of
    adding into PSUM: all-SBUF fp16 operands run the DVE in 2x mode
    (~680ns vs ~1320ns per [128,1024] chunk) and the chain no longer
    serializes ACT behind DVE on PSUM. exp(s+b) == exp(s)*exp(b).
  * Output projection with both heads' O^T stacked into one
    [128, TOK] tile (h1 shifted down via SBUF->SBUF DMA): single
    contraction-128 matmul per chunk — halves out-proj PE time.
  * Softmax reciprocal: denominators staged to SBUF (frees PSUM fast),
    spread across 128 partitions via SB->SB DMA, one parallel DVE
    reciprocal, shuffled back (the naive [1,512] DVE reciprocal is
    3.3us a pop); broadcast via f16 ones-row matmul instead of the
    baseline's fp32 one (4 cy/row -> 1 cy/row).

Device layout (per core):
  QT/KT   (128, 4096) fp16, partitions = [h0 dk(64) | h1 dk(64)]
  Vb      (128, 32, 130) fp16 per 128-tok chunk: [V_h0(64)|1|V_h1(64)|1]
  scores  computed transposed (k on partitions) in PSUM f32; ACT exp ->
          pe fp16; pe *= estr (DVE 2x); PV accumulates O^T + denom row.
  E_h     (2048, 2048) fp16 host-precomputed exp(bias), DMA-streamed.
  OTb     (128, 4096) fp16 stacked normalized O^T for the out-proj.
"""

import numpy as np
from contextlib import ExitStack

import concourse.bass as bass
import concourse.bacc as bacc
import concourse.mybir as mybir
import concourse.tile as tile
from concourse.bass_utils import run_bass_kernel_spmd
from concourse.masks import make_identity

F32 = mybir.dt.float32
F16 = mybir.dt.float16
AF = mybir.ActivationFunctionType

N_CORES = 8
B, S, D = 2, 2048, 1024
H, DK = 16, 64
TOK = B * S              # 4096
NQ = 512                 # q-chunk (matmul free dim)
NQC = S // NQ            # 4 q-chunks per batch
NJ = S // 128            # 16 k-chunks per batch
DCH = D // 128           # 8 contraction chunks

# Set by test.py for profiling; harness leaves these untouched.
TRACE = False
TRACE_DIR = None

_CACHED_NC = None


def _body(ctx: ExitStack, tc, aps):
    nc = tc.nc
    xtb, wq, wk, wv, bq, bk, bv, wo, E0, E1, out = (
        aps["xtb"], aps["wq"], aps["wk"], aps["wv"], aps["bq"], aps["bk"],
        aps["bv"], aps["wo"], aps["E0"], aps["E1"], aps["out"])
    Eh = [E0, E1]

    const = ctx.enter_context(tc.tile_pool(name="const", bufs=1))
    persist = ctx.enter_context(tc.tile_pool(name="persist", bufs=1))
    spool = ctx.enter_context(tc.tile_pool(name="spsum", bufs=2, space="PSUM"))
    opool = ctx.enter_context(tc.tile_pool(name="opsum", bufs=4, space="PSUM"))

    # ---- constants ----
    # priority order for the startup DMAs: the first matmul needs only wq
    # and the t=0 x chunk, so issue those two before everything else (they
    # otherwise share queue bandwidth with wk/wv/wo and the x prefetches,
    # pushing the first matmul out to ~21us)
    xpool = ctx.enter_context(tc.tile_pool(name="xin", bufs=3))
    vtpool = ctx.enter_context(tc.tile_pool(name="vtp", bufs=1))
    w_sb = {}
    w_sb["q"] = const.tile([128, DCH, 128], F16, tag="wq_sb", name="wq_sb")
    nc.sync.dma_start(w_sb["q"][:], wq[:])
    xbt0 = xpool.tile([128, DCH, NQ], F16, tag="xb", name="xbt0")
    nc.sync.dma_start(xbt0[:, 0:4, :], xtb[:, 0:4, 0:NQ])
    nc.sync.dma_start(xbt0[:, 4:8, :], xtb[:, 4:8, 0:NQ])
    # the t=1 chunk otherwise queues behind wk/wv/wo and arrives ~2us late
    xbt1 = xpool.tile([128, DCH, NQ], F16, tag="xb", name="xbt1")
    nc.sync.dma_start(xbt1[:, 0:4, :], xtb[:, 0:4, NQ:2 * NQ])
    nc.sync.dma_start(xbt1[:, 4:8, :], xtb[:, 4:8, NQ:2 * NQ])
    ident = const.tile([128, 128], F32, tag="id")
    make_identity(nc, ident[:])
    for name, ap in (("k", wk), ("v", wv)):
        t = const.tile([128, DCH, 128], F16, tag=f"w{name}")
        nc.sync.dma_start(t[:], ap[:])
        w_sb[name] = t
    b_sb = {}
    for name, ap in (("q", bq), ("k", bk), ("v", bv)):
        t = const.tile([128, 1], F32, tag=f"b{name}")
        nc.sync.dma_start(t[:], ap[:])
        b_sb[name] = t
    wo_sb = const.tile([128, D], F16, tag="wo")
    nc.sync.dma_start(wo_sb[:], wo[:])
    # f32 ones staged: memset can't write f16 directly through walrus
    # (memset_set_value_type); ACT copy converts
    ones_f32 = const.tile([128, 64], F32, tag="ones_f32")
    nc.vector.memset(ones_f32[:], 1.0)
    ones1 = const.tile([1, 64], F16, tag="ones1")
    nc.scalar.copy(ones1[:], ones_f32[0:1, :])

    # ---- persistent activations ----
    QT = persist.tile([128, TOK], F16, tag="QT")
    KT = persist.tile([128, TOK], F16, tag="KT")
    Vb = persist.tile([128, TOK // 128, 130], F16, tag="Vb")
    OTb = persist.tile([128, TOK], F16, tag="OTb")
    OTt = persist.tile([64, TOK], F16, tag="OTt")
    rec = [persist.tile([1, TOK], F16, tag=f"rec{h}", name=f"rec{h}")
           for h in range(2)]
    # softmax denominators: single-partition DVE reciprocal is ~3.3us per
    # [1,512] chunk, so spread the row over 128 partitions via SB->SB DMA
    # (tok = 16p + c per 2048-token half), reciprocal once, shuffle back
    den_sb = [persist.tile([1, TOK], F32, tag=f"den{h}", name=f"den{h}")
              for h in range(2)]
    den128 = [persist.tile([128, TOK // 128], F32, tag=f"d128{h}",
                           name=f"d128{h}") for h in range(2)]
    rec128 = [persist.tile([128, TOK // 128], F16, tag=f"r128{h}",
                           name=f"r128{h}") for h in range(2)]

    # ones columns of V_ext (positions 64 and 129 of each 130-stripe)
    nc.scalar.copy(
        Vb[:].rearrange("p t (g x) -> p t g x", g=2)[:, :, :, 64:65],
        ones_f32[:].rearrange("p (t g x) -> p t g x", t=TOK // 128, g=2))

    # ---- phases 1-2: QKV projections + V transpose ----
    if True:
        VT = vtpool.tile([128, TOK], F32, tag="VT")
        for t in range(TOK // NQ):
            sl = slice(t * NQ, (t + 1) * NQ)
            if t == 0:
                xbt = xbt0
            elif t == 1:
                xbt = xbt1
            else:
                xbt = xpool.tile([128, DCH, NQ], F16, tag="xb")
                nc.sync.dma_start(xbt[:], xtb[:, :, sl])
            for name, dst in (("q", QT), ("k", KT)):
                ps = opool.tile([128, NQ], F32, tag="o", name="ps_proj")
                for d in range(DCH):
                    nc.tensor.matmul(ps[:], w_sb[name][:, d, :], xbt[:, d, :],
                                     start=(d == 0), stop=(d == DCH - 1))
                nc.vector.tensor_scalar_add(dst[:, sl], ps[:], b_sb[name][:])
            ps = opool.tile([128, NQ], F32, tag="o", name="ps_proj")
            for d in range(DCH):
                nc.tensor.matmul(ps[:], w_sb["v"][:, d, :], xbt[:, d, :],
                                 start=(d == 0), stop=(d == DCH - 1))
            nc.vector.tensor_scalar_add(VT[:, sl], ps[:], b_sb["v"][:])

        # V transpose into (tok, hd) chunks, 4 chunks per psum tile
        for g in range(TOK // 512):
            pt = opool.tile([128, 4, 128], F32, tag="o", name="pt_tr")
            for u in range(4):
                nc.tensor.transpose(
                    pt[:, u, :],
                    VT[:, (4 * g + u) * 128:(4 * g + u + 1) * 128],
                    ident[:])
            nc.vector.tensor_copy(
                Vb[:, 4 * g:4 * g + 4, :]
                .rearrange("p u (g x) -> p u g x", g=2)[:, :, :, 0:64],
                pt[:].rearrange("p u (g x) -> p u g x", g=2))

    # ---- phase 3: attention ----
    bpool = ctx.enter_context(tc.tile_pool(name="bin", bufs=3))
    ppool = ctx.enter_context(tc.tile_pool(name="pexp", bufs=6))
    E_r = [Eh[h].rearrange("(j p) q -> p j q", p=128) for h in range(2)]
    for qc in range(NQC):
        O_ps = [[opool.tile([65, NQ], F32, tag="o", name=f"O_ps{hh}{bb}")
                 for bb in range(2)] for hh in range(2)]
        for jq in range(4):  # quarter-stripes of 4 k-chunks
            estr = bpool.tile([128, 4, 2, NQ], F16, tag="b")
            for hh in range(2):
                nc.sync.dma_start(
                    estr[:, :, hh, :],
                    E_r[hh][:, jq * 4:(jq + 1) * 4, qc * NQ:(qc + 1) * NQ])
            for b in range(2):
                q0 = b * S + qc * NQ
                for ji in range(4):
                    j = jq * 4 + ji
                    k0 = b * S + j * 128
                    sps = spool.tile([128, 2, NQ], F32, tag="s")
                    for hh in range(2):
                        nc.tensor.matmul(
                            sps[:, hh, :],
                            KT[64 * hh:64 * hh + 64, k0:k0 + 128],
                            QT[64 * hh:64 * hh + 64, q0:q0 + NQ],
                            start=True, stop=True)
                    pe = ppool.tile([128, 2, NQ], F16, tag="p")
                    nc.scalar.activation(pe[:], sps[:], AF.Exp)
                    # lag bias as an fp16 multiply (DVE 2x mode, all SBUF)
                    nc.vector.tensor_mul(pe[:], pe[:], estr[:, ji, :, :])
                    for hh in range(2):
                        nc.tensor.matmul(
                            O_ps[hh][b][:],
                            Vb[:, b * NJ + j, 65 * hh:65 * hh + 65],
                            pe[:, hh, :],
                            start=(j == 0), stop=(j == NJ - 1))
        for hh in range(2):
            for b in range(2):
                q0 = b * S + qc * NQ
                nc.vector.tensor_copy(den_sb[hh][0:1, q0:q0 + NQ],
                                      O_ps[hh][b][64:65, :])
                if hh == 0:
                    nc.scalar.copy(OTb[0:64, q0:q0 + NQ], O_ps[hh][b][0:64, :])
                else:
                    nc.scalar.copy(OTt[:, q0:q0 + NQ], O_ps[hh][b][0:64, :])
                    # shift h1 rows into partitions 64-127 of OTb
                    nc.sync.dma_start(OTb[64:128, q0:q0 + NQ],
                                      OTt[:, q0:q0 + NQ])

    # reciprocal of the denominators, 128-partition-parallel
    NC16 = S // 128  # 16 columns per 2048-token half
    for hh in range(2):
        for b in range(2):
            hsl = slice(b * S, (b + 1) * S)
            csl = slice(b * NC16, (b + 1) * NC16)
            nc.sync.dma_start(
                den128[hh][:, csl],
                den_sb[hh][0:1, hsl].rearrange("o (p c) -> o p c", p=128))
            with nc.allow_low_precision(reason="softmax recip to f16"):
                nc.vector.reciprocal(rec128[hh][:, csl], den128[hh][:, csl])
            nc.sync.dma_start(
                rec[hh][0:1, hsl].rearrange("o (p c) -> o p c", p=128),
                rec128[hh][:, csl])

    # ---- phase 3b/4: normalize + output projection, pipelined ----
    # emit each chunk's R_ps broadcast + DVE normalize one iteration ahead
    # of its out-proj matmuls, so the PE never sits in the ~0.9us
    # R_ps(c) -> norm(c) -> outproj(c) dependency gap
    def norm_chunk(c):
        sl = slice(c * NQ, (c + 1) * NQ)
        R_ps = opool.tile([128, NQ], F32, tag="o", name="R_ps")
        for h in range(2):
            nc.tensor.matmul(R_ps[64 * h:64 * h + 64, :], ones1[0:1, :],
                             rec[h][0:1, sl], start=True, stop=True)
        nc.vector.tensor_mul(OTb[:, sl], OTb[:, sl], R_ps[:])

    # lookahead 2: with lookahead 1 the last chunks' normalizes queue
    # behind ~2.8us of drains on the in-order DVE and stall the PE
    norm_chunk(0)
    norm_chunk(1)
    for c in range(TOK // NQ):
        if c + 2 < TOK // NQ:
            norm_chunk(c + 2)
        for u in range(4 * c, 4 * c + 4):
            ps = spool.tile([128, 2, NQ], F32, tag="s")
            for half in range(2):
                nc.tensor.matmul(ps[:, half, :],
                                 OTb[:, u * 128:(u + 1) * 128],
                                 wo_sb[:, half * NQ:(half + 1) * NQ],
                                 start=True, stop=True)
            osb = ppool.tile([128, 2, NQ], F16, tag="osb")
            # alternate engines so the drain isn't serialized on ScalarE
            if u % 2 == 0:
                nc.scalar.copy(osb[:], ps[:])
            else:
                nc.vector.tensor_copy(osb[:], ps[:])
            nc.sync.dma_start(out[u * 128:(u + 1) * 128, :],
                              osb[:].rearrange("p g x -> p (g x)"))


def build_program():
    nc = bacc.Bacc("TRN2", target_bir_lowering=False, debug=False,
                   enable_asserts=False, num_devices=N_CORES)
    aps = {}
    specs = [
        ("xtb", (128, DCH, TOK), F16),
        ("wq", (128, DCH, 128), F16), ("wk", (128, DCH, 128), F16),
        ("wv", (128, DCH, 128), F16),
        ("bq", (128, 1), F32), ("bk", (128, 1), F32), ("bv", (128, 1), F32),
        ("wo", (128, D), F16),
        ("E0", (S, S), F16), ("E1", (S, S), F16),
    ]
    for name, shape, dt in specs:
        aps[name] = nc.dram_tensor(name, shape, dt, kind="ExternalInput").ap()
    aps["out"] = nc.dram_tensor("out", (TOK, D), F16,
                                kind="ExternalOutput").ap()
    with tile.TileContext(nc) as tc:
        with ExitStack() as ctx:
            _body(ctx, tc, aps)
    nc.compile()
    return nc


def _get_nc():
    global _CACHED_NC
    if _CACHED_NC is None:
        _CACHED_NC = build_program()
    return _CACHED_NC


def _host_prep(x, lag, wq, bq, wk, bk, wv, bv, wo, bo, lag_bias):
    x = np.ascontiguousarray(np.asarray(x, dtype=np.float32)).reshape(TOK, D)
    xT = x.T  # (D, TOK)
    lag = np.asarray(lag).astype(np.int64)
    ld = np.abs(lag[:, None] - lag[None, :]).astype(np.int64)
    lag_bias = np.asarray(lag_bias, dtype=np.float32)
    scale = np.float32(1.0 / np.sqrt(DK))
    wqs = np.asarray(wq, np.float32) * scale
    bqs = np.asarray(bq, np.float32) * scale
    wkf = np.asarray(wk, np.float32)
    wvf = np.asarray(wv, np.float32)
    wof = np.asarray(wo, np.float32)

    xtb = np.ascontiguousarray(
        xT.reshape(DCH, 128, TOK).transpose(1, 0, 2).astype(np.float16))

    in_maps = []
    for c in range(N_CORES):
        sl = slice(c * 128, (c + 1) * 128)
        in_maps.append({
            "xtb": xtb,
            "wq": np.ascontiguousarray(
                wqs[:, sl].reshape(DCH, 128, 128).transpose(1, 0, 2)
                .astype(np.float16)),
            "wk": np.ascontiguousarray(
                wkf[:, sl].reshape(DCH, 128, 128).transpose(1, 0, 2)
                .astype(np.float16)),
            "wv": np.ascontiguousarray(
                wvf[:, sl].reshape(DCH, 128, 128).transpose(1, 0, 2)
                .astype(np.float16)),
            "bq": np.ascontiguousarray(bqs[sl].reshape(128, 1)),
            "bk": np.ascontiguousarray(
                np.asarray(bk, np.float32)[sl].reshape(128, 1)),
            "bv": np.ascontiguousarray(
                np.asarray(bv, np.float32)[sl].reshape(128, 1)),
            "wo": np.ascontiguousarray(wof[sl, :].astype(np.float16)),
            "E0": np.ascontiguousarray(
                np.exp(lag_bias[2 * c][ld]).astype(np.float16)),
            "E1": np.ascontiguousarray(
                np.exp(lag_bias[2 * c + 1][ld]).astype(np.float16)),
        })
    return in_maps


def kernel(x, lag, wq, bq, wk, bk, wv, bv, wo, bo, lag_bias):
    nc = _get_nc()
    in_maps = _host_prep(x, lag, wq, bq, wk, bk, wv, bv, wo, bo, lag_bias)
    kwargs = {}
    if TRACE:
        kwargs = dict(trace=True, tmpdir=TRACE_DIR)
    res = run_bass_kernel_spmd(nc, in_maps, core_ids=list(range(N_CORES)),
                               **kwargs)
    if TRACE:
        print(f"HW exec time: {res.exec_time_ns} ns")
    total = res.results[0]["out"].astype(np.float32)
    for c in range(1, N_CORES):
        total += res.results[c]["out"].astype(np.float32)
    total += np.asarray(bo, dtype=np.float32)[None, :]
    return total.reshape(B, S, D)
